# revision 18
# baseline (speedup 1.0000x reference)
"""CausalAttention (B=4, T=2048, C=1024, H=16, D=64) on 8 TRN2 NeuronCores.

Sharding: core c -> (batch b = c//2, head-group hg = c%2 covering heads
hg*8..hg*8+7).  Each core computes QKV for its batch restricted to its 8
heads, causal attention, and the output projection for its half of the q
rows (full channel contraction after a pairwise activation exchange).

Device algorithm (per core, fp16 matmul dtype, fp32 PSUM):
  phase A (two t-halves): qkT[j,t] = Wqk^T x^T  (Q,K transposed [ch, T])
           V[t,j] = x Wv   (stored [128t, 8h, 65] with ones in col 0)
           phase A0 runs cb-outer across 8 parallel PSUM banks so the
           first matmul issues as soon as the first x/W tiles land;
           phase A1 is interleaved into the qc0/qc1 attention stream.
  attention: chunks merged in pairs (qc1+qc0, then qc3+qc2) per head
           pair: sT[k,q] = K_h^T Q_h (causal blocks), expT = exp(.125 s)
           (ACT), av[65,q] = V'_h^T expT accumulated over k-tiles
           (row 0 = sumexp), then at = av[1:65] * bcast(1/av[0]).
  exchange: per (chunk, head-pair) a small AllGather ships the peer's
           256 q-columns of at; proj reads own half from SBUF.
  proj:    out rows = my 256 q-cols of each chunk; contraction over all
           1024 channels = 4 own blocks + 4 peer blocks (Wproj rows are
           pre-reordered per core so the code is rank-uniform).
"""
import ml_dtypes
import numpy as np

import concourse.bass as bass
import concourse.tile as tile
from concourse import bacc, mybir
from concourse.bass_utils import run_bass_kernel_spmd

F32 = mybir.dt.float32
AF = mybir.ActivationFunctionType

B, T, C = 4, 2048, 1024
H, D = 16, 64
HL = 8           # heads per core
CL = HL * D      # local channels (512)
CDT = mybir.dt.float16  # matmul compute dtype
QC = 512         # q-chunk width
NQC = T // QC    # 4
KT = 128         # k-tile
N5 = 512         # matmul free-dim / PSUM bank width (fp32)
RH = QC // 2     # per-rank q rows per chunk (256)


def _build():
    nc = bacc.Bacc("TRN2", target_bir_lowering=False, debug=False, num_devices=8)

    xT = nc.dram_tensor("xT", [8, 128, T], CDT, kind="ExternalInput").ap()
    wqk = nc.dram_tensor("wqk", [8, 128, 1024], CDT, kind="ExternalInput").ap()
    wv = nc.dram_tensor("wv", [8, 128, CL], CDT, kind="ExternalInput").ap()
    wproj = nc.dram_tensor("wproj", [8, 128, C], CDT, kind="ExternalInput").ap()
    bias2 = nc.dram_tensor("bias2", [1, C], CDT, kind="ExternalInput").ap()
    ones_r = nc.dram_tensor("ones_r", [1, 128], CDT, kind="ExternalInput").ap()
    masks = nc.dram_tensor("masks", [128, 128], CDT, kind="ExternalInput").ap()
    vones = nc.dram_tensor("vones", [T // 128, 128, HL, D], CDT,
                           kind="ExternalInput").ap()
    out = nc.dram_tensor("out", [T // 2, C], F32, kind="ExternalOutput").ap()

    with tile.TileContext(nc) as tc:
        _emit(nc, tc, xT, wqk, wv, wproj, bias2, ones_r, masks, vones, out)

    nc.compile()
    return nc


def _emit(nc, tc, xT, wqk, wv, wproj, bias2, ones_r, masks, vones, out):
    with (
        tc.tile_pool(name="persist", bufs=1) as pp,
        tc.tile_pool(name="xtp", bufs=16) as xtp,
        tc.tile_pool(name="expp", bufs=4) as expp,
        tc.tile_pool(name="atp", bufs=4) as atp,
        tc.tile_pool(name="nrm", bufs=4) as nrm,
        tc.tile_pool(name="agtp", bufs=2) as agtp,
        tc.tile_pool(name="stg", bufs=2) as stg,
        tc.tile_pool(name="drp", bufs=4, space="DRAM") as drp,
    ):
        # qkT[jt]: channels 128*jt..128*jt+127 (j<512: Q; j>=512: K), [128, T]
        qkT = [pp.tile([128, T], CDT, name=f"qkT{j}") for j in range(8)]
        # VV[tb]: [128 t, HL heads, 128] - col 0 ones (sumexp), 1..63 zero,
        # 64..127 V data: av partition 0 = sumexp, 64..127 = out channels
        VV = [pp.tile([128, HL, 128], CDT, name=f"VV{t}") for t in range(T // 128)]
        wqk_t = [pp.tile([128, 1024], CDT, name=f"wqk{i}") for i in range(8)]
        wv_t = [pp.tile([128, CL], CDT, name=f"wv{i}") for i in range(8)]
        wproj_t8 = [pp.tile([128, C], CDT, name=f"wproj{i}") for i in range(8)]
        bias_t = pp.tile([1, C], CDT, name="bias_t")
        ones_t = pp.tile([1, 128], CDT, name="ones_t")
        mask_t = pp.tile([128, 128], CDT, name="mask_t")
        mask2_t = pp.tile([128, 2, 128], CDT, name="mask2_t")

        with tc.tile_critical():
            rid = nc.sync.partition_id()
            rankoff = (rid % 2) * RH          # my q-column half
            peeroff = ((rid + 1) % 2) * RH    # peer's q-column half
            pioff = ((rid + 1) % 2) * 128     # peer's shard rows in ag

        # initial DMAs: wqk/x interleaved so cb-outer waves start early
        xt_all = {0: [], 1: []}
        for cb in range(8):
            nc.sync.dma_start(out=wqk_t[cb], in_=wqk[cb])
            x_t = xtp.tile([128, 1024], CDT, tag="xt", name=f"xt0_{cb}")
            nc.sync.dma_start(out=x_t, in_=xT[cb, :, 0:1024])
            xt_all[0].append(x_t)
        nc.sync.dma_start(out=mask_t, in_=masks)
        nc.sync.dma_start(out=mask2_t[:, 0, :], in_=masks)
        nc.sync.dma_start(out=mask2_t[:, 1, :], in_=masks)
        nc.sync.dma_start(out=bias_t, in_=bias2)
        nc.sync.dma_start(out=ones_t, in_=ones_r)
        for i in range(8):
            nc.sync.dma_start(out=wv_t[i], in_=wv[i])
        for cb in range(8):
            x_t = xtp.tile([128, 1024], CDT, tag="xt", name=f"xt1_{cb}")
            nc.sync.dma_start(out=x_t, in_=xT[cb, :, 1024:2048])
            xt_all[1].append(x_t)
        for i in range(8):
            nc.sync.dma_start(out=wproj_t8[i], in_=wproj[i])

        at_all, ags = {}, {}

        def emit_qk_wave(pool, tch, jts):
            """qkT groups for jt in jts, cb-outer accumulation."""
            t0 = tch * 1024
            xt = xt_all[tch]
            groups = [(jt, s5) for jt in jts for s5 in range(2)]
            tiles = {}
            for (jt, s5) in groups:
                tiles[(jt, s5)] = pool.tile(
                    [128, N5], F32, tag="pa", name=f"pqk{tch}{jt}{s5}")
            for cb in range(8):
                for (jt, s5) in groups:
                    nc.tensor.matmul(
                        tiles[(jt, s5)], wqk_t[cb][:, jt * 128:(jt + 1) * 128],
                        xt[cb][:, s5 * N5:(s5 + 1) * N5],
                        start=(cb == 0), stop=(cb == 7))
            for (jt, s5) in groups:
                nc.scalar.activation(
                    qkT[jt][:, t0 + s5 * N5: t0 + (s5 + 1) * N5],
                    tiles[(jt, s5)], AF.Copy)

        def emit_v_wave(pool, tch, tbs):
            xt = xt_all[tch]
            tiles = {}
            for tb in tbs:
                tiles[tb] = pool.tile(
                    [128, CL], F32, tag="pa", name=f"pv{tch}{tb}")
            for cb in range(8):
                for tb in tbs:
                    nc.tensor.matmul(
                        tiles[tb], xt[cb][:, tb * 128:(tb + 1) * 128],
                        wv_t[cb], start=(cb == 0), stop=(cb == 7))
            for tb in tbs:
                gtb = tch * 8 + tb
                nc.scalar.activation(
                    VV[gtb][:, :, D:2 * D],
                    tiles[tb].rearrange("p (h d) -> p h d", h=HL), AF.Copy)
                nc.sync.dma_start(out=VV[gtb][:, :, 0:D], in_=vones[gtb])

        # ---- phase A0: t 0..1023, 8-deep PSUM waves, K then V then Q ----
        with tc.tile_pool(name="pa", bufs=8, space="PSUM") as pa:
            emit_qk_wave(pa, 0, [4, 5, 6, 7])     # K first
            emit_v_wave(pa, 0, list(range(8)))    # V for kt 0..7
            for jt in range(4):                   # Q, jt0 first (hp0 needs it)
                emit_qk_wave(pa, 0, [jt])

        with (
            tc.tile_pool(name="ps_s", bufs=2, space="PSUM") as ps_s,
            tc.tile_pool(name="ps_av", bufs=4, space="PSUM") as ps_av,
        ):
            # ---- leftover A0 + A1 groups, interleaved into qc0/qc1 ----
            def il_qk_group(tch, jt, s5):
                xt = xt_all[tch]
                t0 = tch * 1024
                ps = ps_s.tile([128, N5], F32, tag="s",
                               name=f"ilqk{tch}{jt}{s5}")
                for cb in range(8):
                    nc.tensor.matmul(
                        ps, wqk_t[cb][:, jt * 128:(jt + 1) * 128],
                        xt[cb][:, s5 * N5:(s5 + 1) * N5],
                        start=(cb == 0), stop=(cb == 7))
                nc.scalar.activation(
                    qkT[jt][:, t0 + s5 * N5: t0 + (s5 + 1) * N5],
                    ps, AF.Copy)

            def a1_v_group(tb, on_act=True, tch=1):
                xt = xt_all[tch]
                ps = ps_s.tile([128, CL], F32, tag="s", name=f"ilv{tch}{tb}")
                for cb in range(8):
                    nc.tensor.matmul(
                        ps, xt[cb][:, tb * 128:(tb + 1) * 128], wv_t[cb],
                        start=(cb == 0), stop=(cb == 7))
                gtb = tch * 8 + tb
                if on_act:
                    nc.scalar.activation(
                        VV[gtb][:, :, D:2 * D],
                        ps.rearrange("p (h d) -> p h d", h=HL), AF.Copy)
                else:
                    nc.vector.tensor_copy(
                        VV[gtb][:, :, D:2 * D],
                        ps.rearrange("p (h d) -> p h d", h=HL))
                nc.sync.dma_start(out=VV[gtb][:, :, 0:D], in_=vones[gtb])

            # priority order: what win1's later hp blocks need first (tch0),
            # then A1 (tch1: K, V kt8..11, Q) needed by win2
            a1_groups = (
                [(il_qk_group, (1, jt, s5)) for jt in (4, 5, 6, 7)
                   for s5 in range(2)]
                + [(a1_v_group, (tb, True, 1)) for tb in range(4)]
                + [(il_qk_group, (1, jt, s5)) for jt in (0, 1, 2, 3)
                   for s5 in range(2)])
            a1_pos = [0]

            def emit_a1(n):
                for _ in range(n):
                    if a1_pos[0] >= len(a1_groups):
                        return
                    fn, args = a1_groups[a1_pos[0]]
                    a1_pos[0] += 1
                    fn(*args)

            # ---- attention block for (qc, hp): scores+exp+attnv ----
            def emit_attn_block(qc, hp):
                q0 = qc * QC
                nkt = (q0 + QC) // KT
                heads = (2 * hp, 2 * hp + 1)
                av = {h: ps_av.tile([128, N5], F32, tag="av",
                                    name=f"av{qc}_{h}")
                      for h in heads}
                exps = {}

                def emit_scores(kt):
                    k0 = kt * KT
                    est = max(0, k0 - q0)
                    sp = ps_s.tile([128, 2, N5], F32, tag="s",
                                   name=f"s{qc}_{hp}_{kt}")
                    for h in heads:
                        roff = (h % 2) * D
                        nc.tensor.matmul(
                            sp[:, h % 2, est:N5],
                            qkT[4 + h // 2][roff:roff + D, k0:k0 + KT],
                            qkT[h // 2][roff:roff + D, q0 + est:q0 + QC],
                            start=True, stop=True)
                    ex = expp.tile([128, 2, N5], CDT, tag="exp",
                                   name=f"ex{qc}_{hp}_{kt}")
                    nc.scalar.activation(
                        ex[:, :, est:N5], sp[:, :, est:N5],
                        AF.Exp, scale=0.125)
                    if k0 >= q0:  # zero masked part of the diagonal slab
                        nc.vector.tensor_mul(
                            ex[:, :, est:est + KT],
                            ex[:, :, est:est + KT], mask2_t)
                    exps[kt] = ex

                def emit_attnv(kt):
                    k0 = kt * KT
                    cst = max(0, k0 - q0)
                    ex = exps.pop(kt)
                    for h in heads:
                        nc.tensor.matmul(
                            av[h][:, cst:N5], VV[kt][:, h, :],
                            ex[:, h % 2, cst:N5],
                            start=(kt == 0), stop=(kt == nkt - 1))

                emit_scores(0)
                for kt in range(1, nkt):
                    emit_scores(kt)
                    emit_attnv(kt - 1)
                emit_attnv(nkt - 1)
                return av

            def emit_norm(qc, hp, av):
                at = at_all[qc]
                rcs = {}
                for h in (2 * hp, 2 * hp + 1):
                    rc = nrm.tile([1, N5], F32, tag="rc", name=f"rc{qc}_{h}")
                    nc.vector.reciprocal_approx_fast(out=rc, in_=av[h][0:1, :])
                    rcs[h] = rc
                for h in (2 * hp, 2 * hp + 1):
                    rb = nrm.tile([D, N5], F32, tag="rb", name=f"rb{qc}_{h}")
                    nc.gpsimd.partition_broadcast(rb, rcs[h])
                    roff = (h % 2) * D
                    nc.vector.tensor_mul(at[hp][roff:roff + D, :],
                                         av[h][D:2 * D, :], rb)

            def emit_exchange(qc, hp):
                """ship my at[hp] peer-half to the peer (pairwise AG)."""
                ad = drp.tile([128, RH], CDT, tag="ad", name=f"ad{qc}_{hp}")
                nc.sync.dma_start(
                    out=ad, in_=at_all[qc][hp][:, bass.ds(peeroff, RH)])
                ag = drp.tile([2, 128, RH], CDT, tag=f"ag{hp}",
                              name=f"ag{qc}_{hp}")
                nc.gpsimd.collective_compute(
                    "AllGather", mybir.AluOpType.bypass,
                    replica_groups=[[0, 1], [2, 3], [4, 5], [6, 7]],
                    ins=[ad[:]], outs=[ag[:]])
                ags[(qc, hp)] = ag

            proj_agts, proj_sts = {}, {}

            def emit_proj_dmas(qc):
                """fetch the 8 contraction blocks for chunk qc's proj."""
                at = at_all[qc]
                agts = []
                for ci in range(4):  # my 4 channel blocks, from SBUF
                    o = agtp.tile([128, RH], CDT, tag=f"ao{ci}",
                                  name=f"ao{qc}_{ci}")
                    nc.sync.dma_start(
                        out=o, in_=at[ci][:, bass.ds(rankoff, RH)])
                    agts.append(o)
                for ci in range(4):  # peer's 4 channel blocks, from AG
                    p = agtp.tile([128, RH], CDT, tag=f"ap{ci}",
                                  name=f"ap{qc}_{ci}")
                    agr = ags[(qc, ci)].rearrange("r c t -> (r c) t")
                    nc.sync.dma_start(
                        out=p, in_=agr[bass.ds(pioff, 128), :])
                    agts.append(p)
                proj_agts[qc] = agts

            def proj_group_own(qc, tt, jc):
                """open the PSUM group with my 4 local channel blocks."""
                agts = proj_agts[qc]
                pps = ps_s.tile([128, N5], F32, tag="s",
                                name=f"pp{qc}_{tt}_{jc}")
                for n in range(4):
                    nc.tensor.matmul(
                        pps, agts[n][:, tt * 128:(tt + 1) * 128],
                        wproj_t8[n][:, jc * N5:(jc + 1) * N5],
                        start=(n == 0), stop=False)
                return pps

            def proj_group_rest(qc, tt, jc, pps):
                """peer blocks + bias, close group, stage + store."""
                agts = proj_agts[qc]
                if (qc, tt) not in proj_sts:
                    proj_sts[(qc, tt)] = stg.tile(
                        [128, C], F32, tag="stage", name=f"stg{qc}_{tt}")
                st = proj_sts[(qc, tt)]
                for i8 in range(4, 8):
                    nc.tensor.matmul(
                        pps, agts[i8][:, tt * 128:(tt + 1) * 128],
                        wproj_t8[i8][:, jc * N5:(jc + 1) * N5],
                        start=False, stop=False)
                nc.tensor.matmul(
                    pps, ones_t, bias_t[0:1, jc * N5:(jc + 1) * N5],
                    start=False, stop=True)
                nc.vector.tensor_copy(st[:, jc * N5:(jc + 1) * N5], pps)
                nc.sync.dma_start(
                    out=out[qc * RH + tt * 128:qc * RH + (tt + 1) * 128,
                            jc * N5:(jc + 1) * N5],
                    in_=st[:, jc * N5:(jc + 1) * N5])

            def emit_proj_group(qc, tt, jc, own_first=False):
                pps = proj_group_own(qc, tt, jc)
                proj_group_rest(qc, tt, jc, pps)

            # ---- schedule ----
            # window 1: qc1+qc0 attention with A1 interleaved (3 groups/blk)
            for qc in (0, 1):
                at_all[qc] = [atp.tile([128, QC], CDT, tag=f"at{ci}",
                                       name=f"at{qc}_{ci}")
                              for ci in range(4)]
            for hp in range(4):
                av1 = emit_attn_block(1, hp)
                emit_norm(1, hp, av1)
                emit_exchange(1, hp)
                emit_a1(2)
                av0 = emit_attn_block(0, hp)
                emit_norm(0, hp, av0)
                emit_exchange(0, hp)
                emit_a1(1)
            emit_a1(len(a1_groups))  # any leftovers

            # window 2: qc3+qc2 attention with proj0/proj1 interleaved
            for qc in (2, 3):
                at_all[qc] = [atp.tile([128, QC], CDT, tag=f"at{ci}",
                                       name=f"at{qc}_{ci}")
                              for ci in range(4)]
            PG = [(0, 0), (0, 1), (1, 0), (1, 1)]  # (tt, jc) group order
            for hp in range(4):  # alternate qc2 / qc3 blocks
                av2 = emit_attn_block(2, hp)
                emit_norm(2, hp, av2)
                emit_exchange(2, hp)
                if hp == 0:
                    for tb in (4, 5, 6, 7):
                        a1_v_group(tb, on_act=False, tch=1)
                elif hp == 1:
                    emit_proj_dmas(0)
                    emit_proj_group(0, *PG[0])
                    emit_proj_group(0, *PG[1])
                elif hp == 2:
                    emit_proj_dmas(1)
                    emit_proj_group(1, *PG[0])
                    emit_proj_group(1, *PG[1])
                av3 = emit_attn_block(3, hp)
                emit_norm(3, hp, av3)
                emit_exchange(3, hp)
                if hp == 1:
                    emit_proj_group(0, *PG[2])
                    emit_proj_group(0, *PG[3])
                elif hp == 2:
                    emit_proj_group(1, *PG[2])
                    emit_proj_group(1, *PG[3])
            emit_proj_dmas(2)
            emit_proj_dmas(3)
            for tt, jc in PG:
                p2 = proj_group_own(2, tt, jc)
                p3 = proj_group_own(3, tt, jc)
                proj_group_rest(2, tt, jc, p2)
                proj_group_rest(3, tt, jc, p3)


def _prepare_in_maps(x, Wqkv, Wproj, bproj):
    x = np.asarray(x, dtype=np.float32)
    Wqkv = np.asarray(Wqkv, dtype=np.float32)
    Wproj = np.asarray(Wproj, dtype=np.float32)
    bproj = np.asarray(bproj, dtype=np.float32)

    # triangular keep-mask slab: 1 where q >= k (kept), 0 where masked
    k_i = np.arange(128)[:, None]
    q_i = np.arange(128)[None, :]
    masks = np.where(q_i >= k_i, np.float32(1.0), np.float32(0.0))
    masks = np.ascontiguousarray(masks, dtype=np.float32)

    ones_r = np.ones((1, 128), dtype=np.float32)
    # VV filler: col 0 = 1 (sumexp ones), cols 1..63 = 0
    vones = np.zeros((T // 128, 128, HL, D), dtype=np.float32)
    vones[:, :, :, 0] = 1.0

    in_maps = []
    for core in range(8):
        b, hg = core // 2, core % 2
        xT = np.ascontiguousarray(x[b].T).reshape(8, 128, T)
        wq = Wqkv[:, hg * CL:(hg + 1) * CL]
        wk = Wqkv[:, C + hg * CL: C + (hg + 1) * CL]
        wv_ = Wqkv[:, 2 * C + hg * CL: 2 * C + (hg + 1) * CL]
        wqk = np.ascontiguousarray(
            np.concatenate([wq, wk], axis=1)).reshape(8, 128, 1024)
        wv = np.ascontiguousarray(wv_).reshape(8, 128, CL)
        # Wproj rows reordered per core: my 4 channel blocks, then peer's
        wp = np.ascontiguousarray(np.concatenate(
            [Wproj[hg * CL:(hg + 1) * CL],
             Wproj[(1 - hg) * CL:(2 - hg) * CL]], axis=0)).reshape(8, 128, C)
        f16 = np.float16
        in_maps.append({
            "xT": xT.astype(f16), "wqk": wqk.astype(f16),
            "wv": wv.astype(f16), "wproj": wp.astype(f16),
            "bias2": bproj.reshape(1, C).astype(f16),
            "ones_r": ones_r.astype(f16),
            "masks": masks.astype(f16), "vones": vones.astype(f16),
        })
    return in_maps


def _assemble(results):
    full = np.empty((B, T, C), dtype=np.float32)
    for core in range(8):
        b, r = core // 2, core % 2
        o = results[core]["out"]  # [1024, 1024]
        for qc in range(NQC):
            g0 = qc * QC + r * RH
            full[b, g0:g0 + RH] = o[qc * RH:(qc + 1) * RH]
    return full


_NC_CACHE = None


def kernel(x, Wqkv, Wproj, bproj):
    global _NC_CACHE
    if _NC_CACHE is None:
        _NC_CACHE = _build()
    in_maps = _prepare_in_maps(x, Wqkv, Wproj, bproj)
    res = run_bass_kernel_spmd(_NC_CACHE, in_maps, list(range(8)))
    return _assemble(res.results)


# revision 19
# speedup vs baseline: 1.0098x; 1.0098x over previous
"""CausalAttention (B=4, T=2048, C=1024, H=16, D=64) on 8 TRN2 NeuronCores.

Sharding: core c -> (batch b = c//2, head-group hg = c%2 covering heads
hg*8..hg*8+7).  Each core computes QKV for its batch restricted to its 8
heads, causal attention, and the output projection for its half of the q
rows (full channel contraction after a pairwise activation exchange).

Device algorithm (per core, fp16 matmul dtype, fp32 PSUM):
  phase A (two t-halves): qkT[j,t] = Wqk^T x^T  (Q,K transposed [ch, T])
           V[t,j] = x Wv   (stored [128t, 8h, 65] with ones in col 0)
           phase A0 runs cb-outer across 8 parallel PSUM banks so the
           first matmul issues as soon as the first x/W tiles land;
           phase A1 is interleaved into the qc0/qc1 attention stream.
  attention: chunks merged in pairs (qc1+qc0, then qc3+qc2) per head
           pair: sT[k,q] = K_h^T Q_h (causal blocks), expT = exp(.125 s)
           (ACT), av[65,q] = V'_h^T expT accumulated over k-tiles
           (row 0 = sumexp), then at = av[1:65] * bcast(1/av[0]).
  exchange: per (chunk, head-pair) a small AllGather ships the peer's
           256 q-columns of at; proj reads own half from SBUF.
  proj:    out rows = my 256 q-cols of each chunk; contraction over all
           1024 channels = 4 own blocks + 4 peer blocks (Wproj rows are
           pre-reordered per core so the code is rank-uniform).
"""
import ml_dtypes
import numpy as np

import concourse.bass as bass
import concourse.tile as tile
from concourse import bacc, mybir
from concourse.bass_utils import run_bass_kernel_spmd

F32 = mybir.dt.float32
AF = mybir.ActivationFunctionType

B, T, C = 4, 2048, 1024
H, D = 16, 64
HL = 8           # heads per core
CL = HL * D      # local channels (512)
CDT = mybir.dt.float16  # matmul compute dtype
QC = 512         # q-chunk width
NQC = T // QC    # 4
KT = 128         # k-tile
N5 = 512         # matmul free-dim / PSUM bank width (fp32)
RH = QC // 2     # per-rank q rows per chunk (256)


def _build():
    nc = bacc.Bacc("TRN2", target_bir_lowering=False, debug=False, num_devices=8)

    xT = nc.dram_tensor("xT", [8, 128, T], CDT, kind="ExternalInput").ap()
    wqk = nc.dram_tensor("wqk", [8, 128, 1024], CDT, kind="ExternalInput").ap()
    wv = nc.dram_tensor("wv", [8, 128, CL], CDT, kind="ExternalInput").ap()
    wproj = nc.dram_tensor("wproj", [8, 128, C], CDT, kind="ExternalInput").ap()
    bias2 = nc.dram_tensor("bias2", [1, C], CDT, kind="ExternalInput").ap()
    ones_r = nc.dram_tensor("ones_r", [1, 128], CDT, kind="ExternalInput").ap()
    masks = nc.dram_tensor("masks", [128, 128], CDT, kind="ExternalInput").ap()
    vones = nc.dram_tensor("vones", [T // 128, 128, HL, D], CDT,
                           kind="ExternalInput").ap()
    out = nc.dram_tensor("out", [T // 2, C], F32, kind="ExternalOutput").ap()

    with tile.TileContext(nc) as tc:
        _emit(nc, tc, xT, wqk, wv, wproj, bias2, ones_r, masks, vones, out)

    nc.compile()
    return nc


def _emit(nc, tc, xT, wqk, wv, wproj, bias2, ones_r, masks, vones, out):
    with (
        tc.tile_pool(name="persist", bufs=1) as pp,
        tc.tile_pool(name="xtp", bufs=16) as xtp,
        tc.tile_pool(name="expp", bufs=4) as expp,
        tc.tile_pool(name="atp", bufs=4) as atp,
        tc.tile_pool(name="nrm", bufs=4) as nrm,
        tc.tile_pool(name="agtp", bufs=2) as agtp,
        tc.tile_pool(name="stg", bufs=2) as stg,
        tc.tile_pool(name="drp", bufs=4, space="DRAM") as drp,
    ):
        # qkT[jt]: channels 128*jt..128*jt+127 (j<512: Q; j>=512: K), [128, T]
        qkT = [pp.tile([128, T], CDT, name=f"qkT{j}") for j in range(8)]
        # VV[tb]: [128 t, HL heads, 128] - col 0 ones (sumexp), 1..63 zero,
        # 64..127 V data: av partition 0 = sumexp, 64..127 = out channels
        VV = [pp.tile([128, HL, 128], CDT, name=f"VV{t}") for t in range(T // 128)]
        wqk_t = [pp.tile([128, 1024], CDT, name=f"wqk{i}") for i in range(8)]
        wv_t = [pp.tile([128, CL], CDT, name=f"wv{i}") for i in range(8)]
        wproj_t8 = [pp.tile([128, C], CDT, name=f"wproj{i}") for i in range(8)]
        bias_t = pp.tile([1, C], CDT, name="bias_t")
        ones_t = pp.tile([1, 128], CDT, name="ones_t")
        mask_t = pp.tile([128, 128], CDT, name="mask_t")

        with tc.tile_critical():
            rid = nc.sync.partition_id()
            rankoff = (rid % 2) * RH          # my q-column half
            peeroff = ((rid + 1) % 2) * RH    # peer's q-column half
            pioff = ((rid + 1) % 2) * 128     # peer's shard rows in ag

        # initial DMAs: wqk/x interleaved so cb-outer waves start early
        xt_all = {0: [], 1: []}
        for cb in range(8):
            nc.sync.dma_start(out=wqk_t[cb], in_=wqk[cb])
            x_t = xtp.tile([128, 1024], CDT, tag="xt", name=f"xt0_{cb}")
            nc.sync.dma_start(out=x_t, in_=xT[cb, :, 0:1024])
            xt_all[0].append(x_t)
        nc.sync.dma_start(out=mask_t, in_=masks)
        nc.sync.dma_start(out=bias_t, in_=bias2)
        nc.sync.dma_start(out=ones_t, in_=ones_r)
        for i in range(8):
            nc.sync.dma_start(out=wv_t[i], in_=wv[i])
        for cb in range(8):
            x_t = xtp.tile([128, 1024], CDT, tag="xt", name=f"xt1_{cb}")
            nc.sync.dma_start(out=x_t, in_=xT[cb, :, 1024:2048])
            xt_all[1].append(x_t)
        for i in range(8):
            nc.sync.dma_start(out=wproj_t8[i], in_=wproj[i])

        at_all, ags = {}, {}

        def emit_qk_wave(pool, tch, jts):
            """qkT groups for jt in jts, cb-outer accumulation."""
            t0 = tch * 1024
            xt = xt_all[tch]
            groups = [(jt, s5) for jt in jts for s5 in range(2)]
            tiles = {}
            for (jt, s5) in groups:
                tiles[(jt, s5)] = pool.tile(
                    [128, N5], F32, tag="pa", name=f"pqk{tch}{jt}{s5}")
            for cb in range(8):
                for (jt, s5) in groups:
                    nc.tensor.matmul(
                        tiles[(jt, s5)], wqk_t[cb][:, jt * 128:(jt + 1) * 128],
                        xt[cb][:, s5 * N5:(s5 + 1) * N5],
                        start=(cb == 0), stop=(cb == 7))
            for (jt, s5) in groups:
                nc.scalar.activation(
                    qkT[jt][:, t0 + s5 * N5: t0 + (s5 + 1) * N5],
                    tiles[(jt, s5)], AF.Copy)

        def emit_v_wave(pool, tch, tbs):
            xt = xt_all[tch]
            tiles = {}
            for tb in tbs:
                tiles[tb] = pool.tile(
                    [128, CL], F32, tag="pa", name=f"pv{tch}{tb}")
            for cb in range(8):
                for tb in tbs:
                    nc.tensor.matmul(
                        tiles[tb], xt[cb][:, tb * 128:(tb + 1) * 128],
                        wv_t[cb], start=(cb == 0), stop=(cb == 7))
            for tb in tbs:
                gtb = tch * 8 + tb
                nc.scalar.activation(
                    VV[gtb][:, :, D:2 * D],
                    tiles[tb].rearrange("p (h d) -> p h d", h=HL), AF.Copy)
                nc.sync.dma_start(out=VV[gtb][:, :, 0:D], in_=vones[gtb])

        # ---- phase A0: t 0..1023, 8-deep PSUM waves, K then V then Q ----
        with tc.tile_pool(name="pa", bufs=8, space="PSUM") as pa:
            emit_qk_wave(pa, 0, [4, 5, 6, 7])     # K first
            emit_v_wave(pa, 0, list(range(8)))    # V for kt 0..7
            for jt in range(4):                   # Q, jt0 first (hp0 needs it)
                emit_qk_wave(pa, 0, [jt])

        with (
            tc.tile_pool(name="ps_s", bufs=2, space="PSUM") as ps_s,
            tc.tile_pool(name="ps_av", bufs=4, space="PSUM") as ps_av,
        ):
            # ---- leftover A0 + A1 groups, interleaved into qc0/qc1 ----
            def il_qk_group(tch, jt, s5):
                xt = xt_all[tch]
                t0 = tch * 1024
                ps = ps_s.tile([128, N5], F32, tag="s",
                               name=f"ilqk{tch}{jt}{s5}")
                for cb in range(8):
                    nc.tensor.matmul(
                        ps, wqk_t[cb][:, jt * 128:(jt + 1) * 128],
                        xt[cb][:, s5 * N5:(s5 + 1) * N5],
                        start=(cb == 0), stop=(cb == 7))
                nc.scalar.activation(
                    qkT[jt][:, t0 + s5 * N5: t0 + (s5 + 1) * N5],
                    ps, AF.Copy)

            def a1_v_group(tb, on_act=True, tch=1):
                xt = xt_all[tch]
                ps = ps_s.tile([128, CL], F32, tag="s", name=f"ilv{tch}{tb}")
                for cb in range(8):
                    nc.tensor.matmul(
                        ps, xt[cb][:, tb * 128:(tb + 1) * 128], wv_t[cb],
                        start=(cb == 0), stop=(cb == 7))
                gtb = tch * 8 + tb
                if on_act:
                    nc.scalar.activation(
                        VV[gtb][:, :, D:2 * D],
                        ps.rearrange("p (h d) -> p h d", h=HL), AF.Copy)
                else:
                    nc.vector.tensor_copy(
                        VV[gtb][:, :, D:2 * D],
                        ps.rearrange("p (h d) -> p h d", h=HL))
                nc.sync.dma_start(out=VV[gtb][:, :, 0:D], in_=vones[gtb])

            # priority order: what win1's later hp blocks need first (tch0),
            # then A1 (tch1: K, V kt8..11, Q) needed by win2
            a1_groups = (
                [(il_qk_group, (1, jt, s5)) for jt in (4, 5, 6, 7)
                   for s5 in range(2)]
                + [(a1_v_group, (tb, True, 1)) for tb in range(4)]
                + [(il_qk_group, (1, jt, s5)) for jt in (0, 1, 2, 3)
                   for s5 in range(2)])
            a1_pos = [0]

            def emit_a1(n):
                for _ in range(n):
                    if a1_pos[0] >= len(a1_groups):
                        return
                    fn, args = a1_groups[a1_pos[0]]
                    a1_pos[0] += 1
                    fn(*args)

            # ---- attention block for (qc, hp): scores+exp+attnv ----
            def emit_attn_block(qc, hp):
                q0 = qc * QC
                nkt = (q0 + QC) // KT
                heads = (2 * hp, 2 * hp + 1)
                av = {h: ps_av.tile([128, N5], F32, tag="av",
                                    name=f"av{qc}_{h}")
                      for h in heads}
                exps = {}

                def emit_scores(kt):
                    k0 = kt * KT
                    est = max(0, k0 - q0)
                    sp = ps_s.tile([128, 2, N5], F32, tag="s",
                                   name=f"s{qc}_{hp}_{kt}")
                    for h in heads:
                        roff = (h % 2) * D
                        nc.tensor.matmul(
                            sp[:, h % 2, est:N5],
                            qkT[4 + h // 2][roff:roff + D, k0:k0 + KT],
                            qkT[h // 2][roff:roff + D, q0 + est:q0 + QC],
                            start=True, stop=True)
                    ex = expp.tile([128, 2, N5], CDT, tag="exp",
                                   name=f"ex{qc}_{hp}_{kt}")
                    nc.scalar.activation(
                        ex[:, :, est:N5], sp[:, :, est:N5],
                        AF.Exp, scale=0.125)
                    if k0 >= q0:  # zero masked part of the diagonal slab
                        for h in heads:
                            nc.vector.tensor_mul(
                                ex[:, h % 2, est:est + KT],
                                ex[:, h % 2, est:est + KT], mask_t)
                    exps[kt] = ex

                def emit_attnv(kt):
                    k0 = kt * KT
                    cst = max(0, k0 - q0)
                    ex = exps.pop(kt)
                    for h in heads:
                        nc.tensor.matmul(
                            av[h][:, cst:N5], VV[kt][:, h, :],
                            ex[:, h % 2, cst:N5],
                            start=(kt == 0), stop=(kt == nkt - 1))

                emit_scores(0)
                for kt in range(1, nkt):
                    emit_scores(kt)
                    emit_attnv(kt - 1)
                emit_attnv(nkt - 1)
                return av

            def emit_norm(qc, hp, av):
                at = at_all[qc]
                rcs = {}
                for h in (2 * hp, 2 * hp + 1):
                    rc = nrm.tile([1, N5], F32, tag="rc", name=f"rc{qc}_{h}")
                    nc.vector.reciprocal_approx_fast(out=rc, in_=av[h][0:1, :])
                    rcs[h] = rc
                for h in (2 * hp, 2 * hp + 1):
                    rb = nrm.tile([D, N5], F32, tag="rb", name=f"rb{qc}_{h}")
                    nc.gpsimd.partition_broadcast(rb, rcs[h])
                    roff = (h % 2) * D
                    nc.vector.tensor_mul(at[hp][roff:roff + D, :],
                                         av[h][D:2 * D, :], rb)

            def emit_exchange(qc, hp):
                """ship my at[hp] peer-half to the peer (pairwise AG)."""
                ad = drp.tile([128, RH], CDT, tag="ad", name=f"ad{qc}_{hp}")
                nc.sync.dma_start(
                    out=ad, in_=at_all[qc][hp][:, bass.ds(peeroff, RH)])
                ag = drp.tile([2, 128, RH], CDT, tag=f"ag{hp}",
                              name=f"ag{qc}_{hp}")
                nc.gpsimd.collective_compute(
                    "AllGather", mybir.AluOpType.bypass,
                    replica_groups=[[0, 1], [2, 3], [4, 5], [6, 7]],
                    ins=[ad[:]], outs=[ag[:]])
                ags[(qc, hp)] = ag

            proj_agts, proj_sts = {}, {}

            def emit_proj_dmas(qc):
                """fetch the 8 contraction blocks for chunk qc's proj."""
                at = at_all[qc]
                agts = []
                for ci in range(4):  # my 4 channel blocks, from SBUF
                    o = agtp.tile([128, RH], CDT, tag=f"ao{ci}",
                                  name=f"ao{qc}_{ci}")
                    nc.sync.dma_start(
                        out=o, in_=at[ci][:, bass.ds(rankoff, RH)])
                    agts.append(o)
                for ci in range(4):  # peer's 4 channel blocks, from AG
                    p = agtp.tile([128, RH], CDT, tag=f"ap{ci}",
                                  name=f"ap{qc}_{ci}")
                    agr = ags[(qc, ci)].rearrange("r c t -> (r c) t")
                    nc.sync.dma_start(
                        out=p, in_=agr[bass.ds(pioff, 128), :])
                    agts.append(p)
                proj_agts[qc] = agts

            def proj_group_own(qc, tt, jc):
                """open the PSUM group with my 4 local channel blocks."""
                agts = proj_agts[qc]
                pps = ps_s.tile([128, N5], F32, tag="s",
                                name=f"pp{qc}_{tt}_{jc}")
                for n in range(4):
                    nc.tensor.matmul(
                        pps, agts[n][:, tt * 128:(tt + 1) * 128],
                        wproj_t8[n][:, jc * N5:(jc + 1) * N5],
                        start=(n == 0), stop=False)
                return pps

            def proj_group_rest(qc, tt, jc, pps):
                """peer blocks + bias, close group, stage + store."""
                agts = proj_agts[qc]
                if (qc, tt) not in proj_sts:
                    proj_sts[(qc, tt)] = stg.tile(
                        [128, C], F32, tag="stage", name=f"stg{qc}_{tt}")
                st = proj_sts[(qc, tt)]
                for i8 in range(4, 8):
                    nc.tensor.matmul(
                        pps, agts[i8][:, tt * 128:(tt + 1) * 128],
                        wproj_t8[i8][:, jc * N5:(jc + 1) * N5],
                        start=False, stop=False)
                nc.tensor.matmul(
                    pps, ones_t, bias_t[0:1, jc * N5:(jc + 1) * N5],
                    start=False, stop=True)
                nc.vector.tensor_copy(st[:, jc * N5:(jc + 1) * N5], pps)
                nc.sync.dma_start(
                    out=out[qc * RH + tt * 128:qc * RH + (tt + 1) * 128,
                            jc * N5:(jc + 1) * N5],
                    in_=st[:, jc * N5:(jc + 1) * N5])

            def emit_proj_group(qc, tt, jc, own_first=False):
                pps = proj_group_own(qc, tt, jc)
                proj_group_rest(qc, tt, jc, pps)

            # ---- schedule ----
            # window 1: qc1+qc0 attention with A1 interleaved (3 groups/blk)
            for qc in (0, 1):
                at_all[qc] = [atp.tile([128, QC], CDT, tag=f"at{ci}",
                                       name=f"at{qc}_{ci}")
                              for ci in range(4)]
            for hp in range(4):
                av1 = emit_attn_block(1, hp)
                emit_norm(1, hp, av1)
                emit_exchange(1, hp)
                emit_a1(2)
                av0 = emit_attn_block(0, hp)
                emit_norm(0, hp, av0)
                emit_exchange(0, hp)
                emit_a1(1)
            emit_a1(len(a1_groups))  # any leftovers

            # window 2: qc3+qc2 attention with proj0/proj1 interleaved
            for qc in (2, 3):
                at_all[qc] = [atp.tile([128, QC], CDT, tag=f"at{ci}",
                                       name=f"at{qc}_{ci}")
                              for ci in range(4)]
            PG = [(0, 0), (0, 1), (1, 0), (1, 1)]  # (tt, jc) group order
            for hp in range(4):  # alternate qc2 / qc3 blocks
                av2 = emit_attn_block(2, hp)
                emit_norm(2, hp, av2)
                emit_exchange(2, hp)
                if hp == 0:
                    for tb in (4, 5, 6, 7):
                        a1_v_group(tb, on_act=False, tch=1)
                elif hp == 1:
                    emit_proj_dmas(0)
                    emit_proj_group(0, *PG[0])
                    emit_proj_group(0, *PG[1])
                elif hp == 2:
                    emit_proj_dmas(1)
                    emit_proj_group(1, *PG[0])
                    emit_proj_group(1, *PG[1])
                av3 = emit_attn_block(3, hp)
                emit_norm(3, hp, av3)
                emit_exchange(3, hp)
                if hp == 1:
                    emit_proj_group(0, *PG[2])
                    emit_proj_group(0, *PG[3])
                elif hp == 2:
                    emit_proj_group(1, *PG[2])
                    emit_proj_group(1, *PG[3])
            emit_proj_dmas(2)
            for tt, jc in PG:
                emit_proj_group(2, tt, jc)
            emit_proj_dmas(3)
            for tt, jc in PG:
                emit_proj_group(3, tt, jc)


def _prepare_in_maps(x, Wqkv, Wproj, bproj):
    x = np.asarray(x, dtype=np.float32)
    Wqkv = np.asarray(Wqkv, dtype=np.float32)
    Wproj = np.asarray(Wproj, dtype=np.float32)
    bproj = np.asarray(bproj, dtype=np.float32)

    # triangular keep-mask slab: 1 where q >= k (kept), 0 where masked
    k_i = np.arange(128)[:, None]
    q_i = np.arange(128)[None, :]
    masks = np.where(q_i >= k_i, np.float32(1.0), np.float32(0.0))
    masks = np.ascontiguousarray(masks, dtype=np.float32)

    ones_r = np.ones((1, 128), dtype=np.float32)
    # VV filler: col 0 = 1 (sumexp ones), cols 1..63 = 0
    vones = np.zeros((T // 128, 128, HL, D), dtype=np.float32)
    vones[:, :, :, 0] = 1.0

    in_maps = []
    for core in range(8):
        b, hg = core // 2, core % 2
        xT = np.ascontiguousarray(x[b].T).reshape(8, 128, T)
        wq = Wqkv[:, hg * CL:(hg + 1) * CL]
        wk = Wqkv[:, C + hg * CL: C + (hg + 1) * CL]
        wv_ = Wqkv[:, 2 * C + hg * CL: 2 * C + (hg + 1) * CL]
        wqk = np.ascontiguousarray(
            np.concatenate([wq, wk], axis=1)).reshape(8, 128, 1024)
        wv = np.ascontiguousarray(wv_).reshape(8, 128, CL)
        # Wproj rows reordered per core: my 4 channel blocks, then peer's
        wp = np.ascontiguousarray(np.concatenate(
            [Wproj[hg * CL:(hg + 1) * CL],
             Wproj[(1 - hg) * CL:(2 - hg) * CL]], axis=0)).reshape(8, 128, C)
        f16 = np.float16
        in_maps.append({
            "xT": xT.astype(f16), "wqk": wqk.astype(f16),
            "wv": wv.astype(f16), "wproj": wp.astype(f16),
            "bias2": bproj.reshape(1, C).astype(f16),
            "ones_r": ones_r.astype(f16),
            "masks": masks.astype(f16), "vones": vones.astype(f16),
        })
    return in_maps


def _assemble(results):
    full = np.empty((B, T, C), dtype=np.float32)
    for core in range(8):
        b, r = core // 2, core % 2
        o = results[core]["out"]  # [1024, 1024]
        for qc in range(NQC):
            g0 = qc * QC + r * RH
            full[b, g0:g0 + RH] = o[qc * RH:(qc + 1) * RH]
    return full


_NC_CACHE = None


def kernel(x, Wqkv, Wproj, bproj):
    global _NC_CACHE
    if _NC_CACHE is None:
        _NC_CACHE = _build()
    in_maps = _prepare_in_maps(x, Wqkv, Wproj, bproj)
    res = run_bass_kernel_spmd(_NC_CACHE, in_maps, list(range(8)))
    return _assemble(res.results)
